# revision 36
# baseline (speedup 1.0000x reference)
"""Trainium2 Bass kernel for nn_Aggregation (SAN-style local aggregation).

out[n, g*32+cc, h, w] = sum_{kh,kw} input[n, g*32+cc, h-3+kh, w-3+kw] * weight[n, cc, kh*7+kw, h, w]

Sharding: data-parallel over batch N=16 across 8 NeuronCores (2 images/core).

Mode "v2" (default): per-core layout
  partition p = cc*4 + blk   (cc in [0,32): weight channel, blk in [0,4): block of 8 output rows)
  in_nc[p][n, g, rho, w]: rho = 1 + r, r in [0,14) the padded window rows
    (lead row rho=0 and spare row rho=15 absorb column spill), w unpadded.
  Weight is host-permuted to [n, (cc blk), kw, kh, hb, w] with the edge
  output-columns of each kw-tap zeroed host-side (exact: those weights
  multiply conv padding zeros in the reference), so products computed at
  spilled columns are zero.
  DVE computes one fat tensor_tensor per (kw, n): free dims (kh:7, g:8,
  (hb w):256) -- 3 free dims, the TENSOR3D ISA limit; per-instruction
  overhead (~950ns) is amortized over 14336 elems. GPSIMD takes 2 of the
  14 fat multiplies. Products are fp16; the Tensor engine accumulates all
  taps into an fp32 PSUM accumulator via identity matmuls; two fat-prod
  pairs are pre-summed on the DVE to offload the PE.
"""

import numpy as np

N, C, H, W = 16, 256, 32, 32
K, PAD = 7, 3
CC, G = 32, 8
KK = K * K
NCORES = 8
NPC = N // NCORES
BLK, HB = 4, 8
R = 14  # padded window rows per blk
ROWS = 16  # lead pad + 14 + spare
COLP = W + 2 * PAD

MODE = "v2"
_GPS_FATS = ()  # (kw, n) fat-mults offloaded to gpsimd
_MERGES = ()  # ((dst_kwn, src_kwn), ...): dst prod += src prod on DVE
_PROD_BUFS = 3
_SPLIT_TAIL = True  # split the last fat per image into kh halves
_DEBUG_STAGE = False

_cache = {}


def _build_v2():
    import concourse.bacc as bacc
    import concourse.mybir as mybir
    import concourse.tile as tile
    from concourse.bass import AP

    fp32 = mybir.dt.float32
    fp16 = mybir.dt.float16
    mult = mybir.AluOpType.mult
    add = mybir.AluOpType.add

    nc = bacc.Bacc("TRN2", target_bir_lowering=False, debug=False, num_devices=NCORES)
    x = nc.dram_tensor("input", [NPC, C, H, W], fp32, kind="ExternalInput").ap()
    # host-permuted weight: [n, cc*blk, kw, kh, hb, w]
    wt = nc.dram_tensor(
        "wt", [NPC, 128, K, K, HB, W], fp32, kind="ExternalInput"
    ).ap()
    idn = nc.dram_tensor("identity", [128, 128], fp16, kind="ExternalInput").ap()
    zs = nc.dram_tensor("zeros", [2048], fp16, kind="ExternalInput").ap()
    y = nc.dram_tensor("output", [NPC, C, H, W], fp32, kind="ExternalOutput").ap()
    if _DEBUG_STAGE:
        dbg_in = nc.dram_tensor(
            "dbg_in", [128, NPC * G * ROWS * W], fp16, kind="ExternalOutput"
        ).ap()
        dbg_w = nc.dram_tensor(
            "dbg_w", [128, NPC * K * K * HB * W], fp16, kind="ExternalOutput"
        ).ap()
        dbg_p = nc.dram_tensor(
            "dbg_p", [128, K * G * HB * W], fp16, kind="ExternalOutput"
        ).ap()

    IN_PITCH = NPC * G * ROWS * W  # 16384 per-partition elems of in_nc
    W_PITCH = NPC * K * K * HB * W  # 25088

    # (kw, n) fats on gpsimd; the rest on DVE. All of image 0 first so its
    # PSUM evict + output stores overlap image 1's compute instead of piling
    # into the tail.
    GPS = list(_GPS_FATS)
    DVE_ORDER = [
        (3, 0), (2, 0), (4, 0), (1, 0), (5, 0), (0, 0), (6, 0),
        (3, 1), (2, 1), (4, 1), (1, 1), (5, 1), (0, 1), (6, 1),
    ]
    # pairs merged on DVE before the PE sees them: (a, b) -> b += a
    MERGE = dict(_MERGES)
    # PE consumption order (merged-away prods excluded)
    merged_away = set(MERGE.values())
    PE_ORDER = [kn for kn in DVE_ORDER if kn not in merged_away]
    if GPS:
        # gpsimd prods are ready mid-stream; consume them late but not last
        PE_ORDER = [kn for kn in PE_ORDER if kn not in GPS]
        PE_ORDER = PE_ORDER[:-2] + GPS + PE_ORDER[-2:]

    with tile.TileContext(nc) as tc:
        with (
            tc.tile_pool(name="main", bufs=1) as pool,
            tc.tile_pool(name="prod", bufs=_PROD_BUFS) as ppool,
            tc.tile_pool(name="psum", bufs=1, space="PSUM") as pspool,
        ):
            in_nc = pool.tile([128, NPC, G, ROWS, W], fp16)
            w16 = pool.tile([128, NPC, K, K, HB, W], fp16)
            acc = pool.tile([128, NPC * G * HB * W], fp32)
            ident = pool.tile([128, 128], fp16)
            acc_ps = pspool.tile([128, NPC * G * HB * W], fp32)

            nc.sync.dma_start(out=ident[:], in_=idn[:])
            # touch ACT so its table set loads during the DMA wait
            warm = pool.tile([128, 1], fp32)
            nc.scalar.copy(out=warm[:], in_=ident[:, 0:1])

            # row halos (rows 0:4 and 12:16 across all partitions; the input
            # DMAs overwrite the non-halo parts, WAW-ordered by the
            # framework). Lead row 0 / spare row 15 are included: they are
            # read via column spill with zero weights, and 0*NaN = NaN if
            # left uninitialized.
            nc.vector.memset(in_nc[:, :, :, 0:4, :], 0.0)
            nc.vector.memset(in_nc[:, :, :, 12:16, :], 0.0)

            # weight chunks split into kh-halves (two smaller DMAs land on
            # different rings, halving per-chunk wire latency); first chunk
            # ahead of the input stream
            _w_dma(nc, AP, wt, w16, 3, 0, W_PITCH)
            # input cast-DMAs, one per (n, blk): all valid rows at once
            for n in range(NPC):
                for blk in range(BLK):
                    h0 = max(0, blk * HB - PAD)
                    h1 = min(H, blk * HB - PAD + R)
                    r0 = h0 - (blk * HB - PAD)
                    dst = in_nc[blk::BLK, n, :, 1 + r0 : 1 + r0 + (h1 - h0), :]
                    src = x[n].rearrange("(g cc) h w -> cc g h w", g=G)[:, :, h0:h1]
                    nc.gpsimd.dma_start(out=dst, in_=src)
                if n == 0:
                    _w_dma(nc, AP, wt, w16, 2, 0, W_PITCH)
            # weight chunks in compute-consumption order
            for kw, n in [(4, 0), (1, 0), (5, 0), (0, 0), (6, 0),
                          (3, 1), (2, 1), (4, 1), (1, 1), (5, 1), (0, 1), (6, 1)]:
                _w_dma(nc, AP, wt, w16, kw, n, W_PITCH)

            # units: (kw, n, kh0, kh1). The last fat per image is split into
            # kh halves so the PE (and evict/store) can start on the first
            # half while the DVE computes the second.
            def units_of(kw, n):
                if _SPLIT_TAIL and kw in (DVE_ORDER[0][0], DVE_ORDER[-1][0]):
                    return [(kw, n, 0, 4), (kw, n, 4, K)]
                return [(kw, n, 0, K)]

            DVE_UNITS = [u for kn in DVE_ORDER for u in units_of(*kn)]
            PE_UNITS = [u for kn in PE_ORDER for u in units_of(*kn)]
            merged_src = set(MERGE.values())
            PE_UNITS = [u for u in PE_UNITS if (u[0], u[1]) not in merged_src]
            npass_n = {n: sum(1 for u in PE_UNITS if u[1] == n) for n in range(NPC)}

            def fat_aps(kw, n, kh0, kh1, pb):
                v = in_nc[:]
                nk = kh1 - kh0
                in0 = AP(
                    v.tensor,
                    v.offset
                    + n * G * ROWS * W
                    + (ROWS - R - 1) * W
                    - PAD
                    + kw
                    + kh0 * W,
                    [[IN_PITCH, 128], [W, nk], [ROWS * W, G], [1, HB * W]],
                )
                wv = w16[:]
                in1 = AP(
                    wv.tensor,
                    wv.offset
                    + n * K * K * HB * W
                    + kw * K * HB * W
                    + kh0 * HB * W,
                    [[W_PITCH, 128], [HB * W, nk], [0, G], [1, HB * W]],
                )
                po = pb[:]
                outp = AP(
                    po.tensor,
                    po.offset + kh0 * G * HB * W,
                    [[K * G * HB * W, 128], [G * HB * W, nk], [HB * W, G], [1, HB * W]],
                )
                return in0, in1, outp

            prods = {}
            unit_done = set()
            dbg_prod = {}

            def emit_unit(kw, n, kh0, kh1, eng):
                if (kw, n) not in prods:
                    if (kw, n) in GPS:
                        pb = pool.tile(
                            [128, K, G * HB * W], fp16, tag=f"gps{kw}_{n}"
                        )
                    else:
                        pb = ppool.tile([128, K, G * HB * W], fp16)
                    prods[(kw, n)] = pb
                pb = prods[(kw, n)]
                in0, in1, outp = fat_aps(kw, n, kh0, kh1, pb)
                eng.tensor_tensor(out=outp, in0=in0, in1=in1, op=mult)
                unit_done.add((kw, n, kh0, kh1))
                if _DEBUG_STAGE and (kw, n) == (3, 0) and not dbg_prod:
                    dbg_prod[0] = True
                    nc.sync.dma_start(
                        out=dbg_p, in_=pb[:].rearrange("p k f -> p (k f)")
                    )

            # gpsimd fats first in its program order (after its DMAs)
            for kw, n in GPS:
                for u in units_of(kw, n):
                    emit_unit(*u, nc.gpsimd)

            pass_idx = {0: 0, 1: 0}

            def pe_pass(kw, n, kh0, kh1):
                pb = prods[(kw, n)]
                pf = pb[:].rearrange("p k f -> p (k f)")
                i = pass_idx[n]
                for kh in range(kh0, kh1):
                    for b in range(4):
                        nc.tensor.matmul(
                            out=acc_ps[:, n * 2048 + b * 512 : n * 2048 + (b + 1) * 512],
                            lhsT=ident[:],
                            rhs=pf[:, kh * 2048 + b * 512 : kh * 2048 + (b + 1) * 512],
                            start=(i == 0 and kh == kh0),
                            stop=(i == npass_n[n] - 1 and kh == kh1 - 1),
                        )
                pass_idx[n] += 1

            def evict_store(n):
                # PSUM -> SBUF quarters alternating DVE/ACT, stores per (n,g)
                for q in range(4):
                    eng = nc.vector.tensor_copy if q % 2 == 0 else nc.scalar.copy
                    lo = n * 2048 + q * 512
                    eng(out=acc[:, lo : lo + 512], in_=acc_ps[:, lo : lo + 512])
                    for g in (2 * q, 2 * q + 1):
                        dsty = y[n].rearrange(
                            "(g cc) (blk hb) w -> g cc blk (hb w)", g=G, blk=BLK
                        )
                        deng = nc.sync if g % 2 == 0 else nc.scalar
                        deng.dma_start(
                            out=dsty[g],
                            in_=acc[:, n * 2048 + g * 256 : n * 2048 + (g + 1) * 256],
                        )

            emitted = 0
            for kw, n, kh0, kh1 in DVE_UNITS:
                emit_unit(kw, n, kh0, kh1, nc.vector)
                if kh1 == K and (kw, n) in MERGE:
                    src = prods[MERGE[(kw, n)]]
                    dst = prods[(kw, n)]
                    nc.vector.tensor_tensor(
                        out=dst[:], in0=dst[:], in1=src[:], op=add
                    )
                # interleave PE passes as their units complete in program
                # order; emit each image's evict+stores right after its
                # final pass so they overlap the other image's compute
                while emitted < len(PE_UNITS):
                    u = PE_UNITS[emitted]
                    if u not in unit_done:
                        break
                    pe_pass(*u)
                    emitted += 1
                    if pass_idx[u[1]] == npass_n[u[1]]:
                        evict_store(u[1])
            assert emitted == len(PE_UNITS), (emitted, len(PE_UNITS))

            if _DEBUG_STAGE:
                nc.sync.dma_start(
                    out=dbg_in, in_=in_nc[:].rearrange("p n g r w -> p (n g r w)")
                )
                nc.sync.dma_start(
                    out=dbg_w, in_=w16[:].rearrange("p n a b h w -> p (n a b h w)")
                )



    nc.compile()
    return nc


def _w_dma(nc, AP, wt, w16, kw, n, W_PITCH):
    K_, HB_, W_ = 7, 8, 32
    for kh0, kh1 in ((0, 4), (4, 7)):
        base = n * K_ * K_ * HB_ * W_ + kw * K_ * HB_ * W_ + kh0 * HB_ * W_
        dst = AP(
            w16[:].tensor,
            w16[:].offset + base,
            [[W_PITCH, 128], [HB_ * W_, kh1 - kh0], [1, HB_ * W_]],
        )
        src = AP(
            wt.tensor,
            n * 128 * K_ * K_ * HB_ * W_ + kw * K_ * HB_ * W_ + kh0 * HB_ * W_,
            [[K_ * K_ * HB_ * W_, 128], [HB_ * W_, kh1 - kh0], [1, HB_ * W_]],
        )
        nc.gpsimd.dma_start(out=dst, in_=src)


def _get_nc(mode=None):
    mode = mode or MODE
    if mode not in _cache:
        if mode == "v2":
            _cache[mode] = _build_v2()
        else:
            raise ValueError(mode)
    return _cache[mode]


def _prep_weight(weight):
    # [N, CC, KK, H, W] -> [N, (cc blk), kw, kh, hb, w], edge out-columns of
    # each kw zeroed (exact: they multiply conv-padding zeros)
    w = weight.reshape(N, CC, K, K, BLK, HB, W)  # [n, cc, kh, kw, blk, hb, w]
    w = np.ascontiguousarray(w.transpose(0, 1, 4, 3, 2, 5, 6))
    # -> [n, cc, blk, kw, kh, hb, w]
    for kw in range(K):
        if kw < PAD:
            w[:, :, :, kw, :, :, 0 : PAD - kw] = 0.0
        elif kw > PAD:
            w[:, :, :, kw, :, :, W + PAD - kw : W] = 0.0
    return w.reshape(N, 128, K, K, HB, W)


def kernel(input_, weight, _trace=False, _mode=None):
    from concourse.bass_utils import run_bass_kernel_spmd

    nc = _get_nc(_mode)
    input_ = np.ascontiguousarray(input_, dtype=np.float32)
    weight = np.ascontiguousarray(weight, dtype=np.float32)
    wt = _prep_weight(weight)
    eye = np.eye(128, dtype=np.float16)
    zeros = np.zeros(2048, dtype=np.float16)
    in_maps = [
        {
            "input": input_[i * NPC : (i + 1) * NPC],
            "wt": wt[i * NPC : (i + 1) * NPC],
            "identity": eye,
            "zeros": zeros,
        }
        for i in range(NCORES)
    ]
    res = run_bass_kernel_spmd(nc, in_maps, list(range(NCORES)), trace=_trace)
    _cache["last_result"] = res
    out = np.concatenate([res.results[i]["output"] for i in range(NCORES)], axis=0)
    return out
